# revision 2
# baseline (speedup 1.0000x reference)
"""MoE layer (top-2 of 8 experts, selection shared across tokens) on 8 TRN2 cores.

Math (faithful to the reference):
    gates = softmax(x @ W_gate + b_gate)          [N, 8]
    idx0  = top-2 expert indices of token 0       [2]
    s     = per-token top-2 gate VALUES (desc)    [N, 2]
    out   = s0 * (x @ W[A] + b[A]) + s1 * (x @ W[B] + b[B])

Strategy: gating + top-2 is 0.2% of the FLOPs -> computed on host.  The two
active expert matmuls (275 GFLOP) are data-parallel sharded over tokens across
8 cores; expert weights are replicated.  Matmuls run in float32r (full PE rate
at free-dim >= 256, ~tf32 precision) accumulating fp32 in PSUM.
"""

import functools
import os

import numpy as np

import concourse.bass as bass
import concourse.mybir as mybir
import concourse.tile as tile
from concourse import bacc
from concourse.bass_utils import run_bass_kernel_spmd

N_CORES = 8
N, D_IN, D_HID = 16384, 2048, 2048
NT = N // N_CORES            # tokens per core
KP = 128                     # contraction chunk = partition dim
KCH = D_IN // KP             # 16 K-chunks
NB = 512                     # output column block (1 PSUM bank of fp32)
NBLK = D_HID // NB           # 4 output blocks
TQ = 256                     # token slice per x-stream piece
NQ = NT // TQ                # 8 slices
MPQ = TQ // 128              # m-tiles per slice

F32 = mybir.dt.float32
F32R = mybir.dt.float32r

# Filled by test harness inspection: last BassKernelResults from a run.
LAST_RESULT = None


@functools.lru_cache(maxsize=1)
def _build():
    nc = bacc.Bacc("TRN2", target_bir_lowering=False, debug=False)
    xT = nc.dram_tensor("xT", [D_IN, NT], F32R, kind="ExternalInput")
    wa = nc.dram_tensor("wa", [D_IN, D_HID], F32R, kind="ExternalInput")
    wb = nc.dram_tensor("wb", [D_IN, D_HID], F32R, kind="ExternalInput")
    bp = nc.dram_tensor("bp", [2, D_HID], F32R, kind="ExternalInput")
    sT = nc.dram_tensor("sT", [2, NT], F32R, kind="ExternalInput")
    sC = nc.dram_tensor("sC", [NT, 2], F32, kind="ExternalInput")
    out = nc.dram_tensor("out", [NT, D_HID], F32, kind="ExternalOutput")

    MULT = mybir.AluOpType.mult
    ADD = mybir.AluOpType.add

    with tile.TileContext(nc) as tc:
        with (
            tc.tile_pool(name="cst", bufs=1) as cst,
            tc.tile_pool(name="wp", bufs=2) as wp,
            tc.tile_pool(name="xp", bufs=2) as xp,
            tc.tile_pool(name="ep", bufs=3) as ep,
            tc.tile_pool(name="ps", bufs=2, space=bass.MemorySpace.PSUM) as ps,
        ):
            sT_sb = cst.tile([2, NT], F32R)
            nc.sync.dma_start(sT_sb[:], sT[:])
            bp_sb = cst.tile([2, D_HID], F32R)
            nc.sync.dma_start(bp_sb[:], bp[:])
            # per-token scores, partition-major: sC_sb[p, m, s] = s_{m*128+p, s}
            sC_sb = cst.tile([128, NT // 128, 2], F32)
            nc.sync.dma_start(sC_sb[:], sC.rearrange("(m p) s -> p m s", p=128))

            for nb in range(NBLK):
                nb_sl = bass.ts(nb, NB)
                w_t = {}
                for e, wd in enumerate((wa, wb)):
                    for k in range(KCH):
                        t = wp.tile([KP, NB], F32R, tag=f"w{e}_{k}")
                        nc.sync.dma_start(t[:], wd[k * KP:(k + 1) * KP, nb_sl])
                        w_t[e, k] = t
                for q in range(NQ):
                    x_t = []
                    for k in range(KCH):
                        t = xp.tile([KP, TQ], F32R, tag=f"x{k}")
                        nc.sync.dma_start(
                            t[:], xT[k * KP:(k + 1) * KP, q * TQ:(q + 1) * TQ]
                        )
                        x_t.append(t)
                    for mi in range(MPQ):
                        mg = q * MPQ + mi
                        pa = ps.tile([128, NB], F32, tag="pa")
                        pb = ps.tile([128, NB], F32, tag="pb")
                        pc = ps.tile([128, NB], F32, tag="pc")
                        # bias outer-product: pc[m,o] = s0[m]*bA[o] + s1[m]*bB[o]
                        nc.tensor.matmul(
                            pc[:],
                            sT_sb[:, bass.ts(mg, 128)],
                            bp_sb[:, nb_sl],
                            start=True,
                            stop=True,
                        )
                        for k in range(KCH):
                            xk = x_t[k][:, bass.ts(mi, 128)]
                            nc.tensor.matmul(
                                pa[:], xk, w_t[0, k][:],
                                start=(k == 0), stop=(k == KCH - 1),
                            )
                            nc.tensor.matmul(
                                pb[:], xk, w_t[1, k][:],
                                start=(k == 0), stop=(k == KCH - 1),
                            )
                        s0 = sC_sb[:, mg, 0:1]
                        s1 = sC_sb[:, mg, 1:2]
                        # t1 = pa * s0   (ACT: PSUM read, per-partition scale)
                        t1 = ep.tile([128, NB], F32, tag="t1")
                        nc.scalar.activation(
                            t1[:], pa[:], mybir.ActivationFunctionType.Copy,
                            scale=s0,
                        )
                        # t2 = pb * s1 + t1   (DVE: one PSUM input)
                        t2 = ep.tile([128, NB], F32, tag="t2")
                        nc.vector.scalar_tensor_tensor(
                            t2[:], pb[:], s1, t1[:], op0=MULT, op1=ADD
                        )
                        # o = t2 + pc        (DVE: one PSUM input)
                        o = ep.tile([128, NB], F32, tag="o")
                        nc.vector.tensor_add(o[:], t2[:], pc[:])
                        nc.sync.dma_start(out[bass.ts(mg, 128), nb_sl], o[:])

    nc.compile()
    return nc


def _host_gating(x, W_gate, b_gate):
    logits = x @ W_gate + b_gate                       # [N, 8] fp32
    m = logits.max(axis=1, keepdims=True)
    e = np.exp(logits - m)
    gates = e / e.sum(axis=1, keepdims=True)
    idx0 = np.argsort(-gates[0], kind="stable")[:2]    # token-0 top-2 experts
    scores = -np.sort(-gates, axis=1)[:, :2]           # per-token top-2 values
    return idx0, np.ascontiguousarray(scores)


def kernel(x, W_experts, b_experts, W_gate, b_gate):
    global LAST_RESULT
    x = np.ascontiguousarray(np.asarray(x, dtype=np.float32))
    W_experts = np.asarray(W_experts, dtype=np.float32)
    b_experts = np.asarray(b_experts, dtype=np.float32)
    W_gate = np.asarray(W_gate, dtype=np.float32)
    b_gate = np.asarray(b_gate, dtype=np.float32)

    idx0, scores = _host_gating(x, W_gate, b_gate)
    wa = np.ascontiguousarray(W_experts[idx0[0]])       # [D_IN, D_HID]
    wb = np.ascontiguousarray(W_experts[idx0[1]])
    bp = np.ascontiguousarray(b_experts[idx0])          # [2, D_HID]

    xT_full = np.ascontiguousarray(x.T)                 # [D_IN, N]

    nc = _build()
    in_maps = []
    for c in range(N_CORES):
        sl = slice(c * NT, (c + 1) * NT)
        in_maps.append(
            {
                "xT": np.ascontiguousarray(xT_full[:, sl]),
                "wa": wa,
                "wb": wb,
                "bp": bp,
                "sT": np.ascontiguousarray(scores[sl].T),
                "sC": np.ascontiguousarray(scores[sl]),
            }
        )

    res = run_bass_kernel_spmd(nc, in_maps, list(range(N_CORES)))
    LAST_RESULT = res
    return np.concatenate([r["out"] for r in res.results], axis=0)
